# revision 1
# baseline (speedup 1.0000x reference)
"""Trainium2 Bass kernel for nn_CriticNetwork (gnn_message_passing).

Key mathematical simplification (verified numerically against the
reference): the reference broadcasts edge_index to (B, 2, E) and
reshapes to (2, B*E).  Row-major reshape interleaves the src/dst
blocks so the resulting src and dst arrays are ELEMENTWISE EQUAL --
every edge is a self-edge v->v.  With GCN normalization
(deg = 1 + 2*count(v), each self-edge contributes x[v]/deg, plus the
explicit self-loop) the aggregate is exactly deg * x[v]/deg = x[v].
Both GCNConv layers therefore collapse to plain linear layers:

    x = relu(x @ W1 + b1); x = relu(x @ W2 + b2)
    node_avg[b] = mean_n(x[b, n] @ node_fc_W) + node_fc_b
    col path is a plain 2-layer MLP; final head is a tiny [4,2] MLP.

Since node_fc / col_W2 are applied linearly after the last relu, the
device only needs per-(batch-slice) SUMS of the hidden activations:
each core processes 25000 nodes (half a batch) + 500 col rows and
returns two small accumulator vectors; the host applies the final
(tiny) linear head.

Device layout per core:
  xT_packed [128, 12500]: rows 0-63  = 64 features of nodes [0, 12500)
                          rows 64-127 = 64 features of nodes [12500, 25000)
  L1 matmul: lhsT = blockdiag(W1, W1) [128, 32] -> h1.T bands [32, 512]
  4 L1 matmuls stack bands in one PSUM bank -> [128, 512]
  relu (ScalarE, bias fused) -> SBUF
  L2 matmul: lhsT = blockdiag(W2 x8) [128, 128] -> [128, 512] PSUM
  relu + accumulate (ScalarE accum_out = per-partition row sum)
  final: reduce accum columns -> node_acc [128, 1] (8 bands of 16)

All constants (weights, biases, col features) ship in ONE packed DMA
("wpack") and a zero-valued warmup matmul consumes it first: the PE
LDWEIGHTS instruction can carry only ONE semaphore wait, so every real
matmul must depend on at most one un-synced DMA lane (its x chunk).
"""

import ml_dtypes
import numpy as np

import concourse.bacc as bacc
import concourse.bass as bass
import concourse.mybir as mybir
import concourse.tile as tile
from concourse.bass_utils import run_bass_kernel_spmd

P = 128
N_CORES = 8
B, N, F_NODE, H = 4, 50000, 64, 16
NODES_PER_CORE = (B * N) // N_CORES        # 25000
COLS = NODES_PER_CORE // 2                 # 12500 packed columns (2 nodes/col)
MM = 512                                   # fp32 matmul max moving free dim
SUPER = 4 * MM                             # 2048 columns per PSUM-bank group
N_CHUNKS = (COLS + SUPER - 1) // SUPER     # 7 (6 full + 212-col tail)
C, F_COL = 1000, 32
COLN = (B * C) // N_CORES                  # 500 col rows per core

# wpack column layout
W1_OFF = 0                                  # [128, 32] blockdiag(W1, W1)
W2_OFF = W1_OFF + 2 * H                     # [128, 128] blockdiag(W2 x8)
B1_OFF = W2_OFF + P                         # [128, 1] b1 tiled x8
B2_OFF = B1_OFF + 1                         # [128, 1] b2 tiled x8
CW1_OFF = B2_OFF + 1                        # [32, 16] col_W1 (rows 0-31)
CB1_OFF = CW1_OFF + H                       # [16, 1] col_b1 (rows 0-15)
ZPAD_OFF = CB1_OFF + 1                      # [128, 1] zeros (warmup operand)
COLT_OFF = ZPAD_OFF + 1                     # [32, 500] colT (rows 0-31)
NW = COLT_OFF + COLN                        # 680

DT = mybir.dt.bfloat16                     # matmul-operand dtype on device
NPDT = ml_dtypes.bfloat16

PROFILE = False        # set True (e.g. from test.py) to collect NTFF timing
CHECK_WAITS = True     # build-time guard: one semaphore wait per compute inst
LAST_EXEC_TIME_NS = None
LAST_RESULTS = None

_NC_CACHE = {}


def _build_nc(relu1_on_dve=True):
    f32 = mybir.dt.float32
    Relu = mybir.ActivationFunctionType.Relu
    # Bacc (not raw Bass): its finalize() runs move_matmul_waits_to_-
    # ldweights + generate_event_semaphores, which legalize schedules
    # against the TRN2 one-semaphore-wait-per-instruction limit.
    nc = bacc.Bacc("TRN2")

    xT = nc.dram_tensor("xT", [P, COLS], DT, kind="ExternalInput")
    wpack = nc.dram_tensor("wpack", [P, NW], DT, kind="ExternalInput")
    node_acc = nc.dram_tensor("node_acc", [P, 1], f32, kind="ExternalOutput")
    col_acc = nc.dram_tensor("col_acc", [H, 1], f32, kind="ExternalOutput")

    with tile.TileContext(nc) as tc:
        with (
            tc.tile_pool(name="consts", bufs=1) as consts,
            tc.tile_pool(name="xin", bufs=4) as xin,
            tc.tile_pool(name="work", bufs=2) as work,
            tc.tile_pool(name="outp", bufs=1) as outp,
            tc.tile_pool(name="psum", bufs=1, space="PSUM") as psum,
        ):
            wp = consts.tile([P, NW], DT)
            nc.sync.dma_start(wp[:], wpack[:])
            w1_t = wp[:, W1_OFF:W1_OFF + 2 * H]
            w2_t = wp[:, W2_OFF:W2_OFF + P]
            b1_t = wp[:, B1_OFF:B1_OFF + 1]
            b2_t = wp[:, B2_OFF:B2_OFF + 1]
            cw1_t = wp[:F_COL, CW1_OFF:CW1_OFF + H]
            cb1_t = wp[:H, CB1_OFF:CB1_OFF + 1]
            zc_t = wp[:, ZPAD_OFF:ZPAD_OFF + 1]
            colT_t = wp[:F_COL, COLT_OFF:COLT_OFF + COLN]

            # Zero stats ON the engine that will accumulate into it (same-
            # engine WAW needs no cross-engine wait).  Reading wpack here
            # also syncs that engine with the wpack DMA lane up front.
            # zeros path: everything post-PE lives on DVE and the Scalar
            # engine is left completely idle (no ACT_TABLE_LOAD either).
            stats = outp.tile([P, N_CHUNKS + 1], f32)
            if relu1_on_dve:
                nc.vector.tensor_scalar_mul(stats[:], wp[:, :N_CHUNKS + 1], 0.0)
            else:
                nc.scalar.mul(stats[:], wp[:, :N_CHUNKS + 1], 0.0)

            # Persistent PSUM tiles (allocated once, manually alternated):
            # a per-chunk pool tile would get a slot-recycle writer guard,
            # an extra PE-sem wait on the first matmul of each chunk -- and
            # the PE LDWEIGHTS instruction can carry only ONE wait.
            NBUF = 3
            ps1_t = [psum.tile([P, MM], f32, tag=f"ps1_{k}", name=f"ps1_{k}")
                     for k in range(NBUF)]
            ps2_t = [psum.tile([P, MM], f32, tag=f"ps2_{k}", name=f"ps2_{k}")
                     for k in range(NBUF)]
            h1r_t = [work.tile([P, MM], DT, tag=f"h1r_{k}", name=f"h1r_{k}")
                     for k in range(NBUF)]
            scr_t = [work.tile([P, MM], DT, tag=f"scr_{k}", name=f"scr_{k}")
                     for k in range(NBUF)]

            # Warmup matmul: syncs PE with the wpack DMA using a single
            # wait, so every later matmul has the wpack lane subsumed.
            # Reads the zero pad column -> contributes exactly 0.0 to
            # stats' spare column (kept live through that write).
            psd = psum.tile([1, 1], f32, tag="psd")
            nc.tensor.matmul(psd[0:1, 0:1], zc_t, zc_t, start=True, stop=True)
            if relu1_on_dve:
                nc.vector.tensor_copy(stats[0:1, N_CHUNKS:N_CHUNKS + 1],
                                      psd[0:1, 0:1])
            else:
                nc.scalar.copy(stats[0:1, N_CHUNKS:N_CHUNKS + 1], psd[0:1, 0:1])

            for s in range(N_CHUNKS):
                c0 = s * SUPER
                cols = min(SUPER, COLS - c0)
                nb = (cols + MM - 1) // MM
                act_w = cols if nb == 1 else cols // nb
                assert act_w * nb == cols, (s, cols, nb)

                x_t = xin.tile([P, SUPER], DT, tag="x")
                nc.sync.dma_start(x_t[:, :cols], xT[:, c0:c0 + cols])

                ps1 = ps1_t[s % NBUF]
                for bnd in range(nb):
                    w = min(MM, cols - bnd * MM)
                    nc.tensor.matmul(
                        ps1[32 * bnd:32 * bnd + 32, :w],
                        w1_t,
                        x_t[:, bnd * MM:bnd * MM + w],
                        start=True, stop=True,
                        tile_position=(0, 32 * bnd),
                    )
                used = 32 * nb

                h1r = h1r_t[s % NBUF]
                if relu1_on_dve:
                    # b1 is structurally zero (setup_inputs uses
                    # jnp.zeros), so relu1 is a plain max with an
                    # immediate -- keeps DVE free of a wpack-DMA wait.
                    nc.vector.tensor_scalar_max(
                        h1r[:used, :act_w], ps1[:used, :act_w], 0.0)
                else:
                    nc.scalar.activation(
                        h1r[:used, :act_w], ps1[:used, :act_w], Relu,
                        bias=b1_t[:used, :],
                    )

                ps2 = ps2_t[s % NBUF]
                nc.tensor.matmul(
                    ps2[:used, :act_w],
                    w2_t[:used, :used],
                    h1r[:used, :act_w],
                    start=True, stop=True,
                )
                scr = scr_t[s % NBUF]
                if relu1_on_dve:
                    # b2 structurally zero: relu2 + row-sum in one DVE op.
                    nc.vector.tensor_scalar(
                        scr[:used, :act_w], ps2[:used, :act_w], 0.0, 0.0,
                        mybir.AluOpType.max, mybir.AluOpType.add,
                        accum_out=stats[:used, s:s + 1],
                    )
                else:
                    nc.scalar.activation(
                        scr[:used, :act_w], ps2[:used, :act_w], Relu,
                        bias=b2_t[:used, :],
                        accum_out=stats[:used, s:s + 1],
                    )

            # column-features path (tiny): h = relu(col @ col_W1 + col_b1)
            psc = psum.tile([H, COLN], f32, tag="psc")
            nc.tensor.matmul(psc[:, :], cw1_t, colT_t, start=True, stop=True)
            colscr = outp.tile([H, COLN], f32)
            col_sb = outp.tile([H, 1], f32)
            if relu1_on_dve:
                # col_b1 structurally zero as well.
                nc.vector.tensor_scalar(
                    colscr[:], psc[:], 0.0, 0.0,
                    mybir.AluOpType.max, mybir.AluOpType.add,
                    accum_out=col_sb[:])
            else:
                nc.scalar.activation(colscr[:], psc[:], Relu,
                                     bias=cb1_t, accum_out=col_sb[:])

            node_sb = outp.tile([P, 1], f32)
            nc.vector.tensor_reduce(node_sb[:], stats[:],
                                    axis=mybir.AxisListType.X,
                                    op=mybir.AluOpType.add)
            nc.sync.dma_start(node_acc[:], node_sb[:])
            nc.sync.dma_start(col_acc[:], col_sb[:])

    nc.finalize()

    # Verify the legalization: at most one wait per instruction
    # (InstEventSemaphore may carry two).
    if CHECK_WAITS:
        for blk in nc.m.functions[0].blocks:
            for inst in blk.instructions:
                si = inst.sync_info
                nwait = len(si.on_wait) if si and si.on_wait else 0
                limit = 2 if type(inst).__name__ in (
                    "InstEventSemaphore", "InstDrain", "InstDMACopy") else 1
                assert nwait <= limit, (
                    inst.name, type(inst).__name__,
                    [w.ant_name for w in si.on_wait])
    return nc


def _get_nc(relu1_on_dve=True):
    key = ("nc", relu1_on_dve)
    if key not in _NC_CACHE:
        _NC_CACHE[key] = _build_nc(relu1_on_dve)
    return _NC_CACHE[key]


def _prep_in_maps(node_features, col_features, W1, b1, W2, b2, col_W1, col_b1):
    x = np.ascontiguousarray(node_features, dtype=np.float32).reshape(B * N, F_NODE)
    colf = np.ascontiguousarray(col_features, dtype=np.float32).reshape(B * C, F_COL)

    W1 = np.asarray(W1, np.float32)
    W2 = np.asarray(W2, np.float32)
    wpack = np.zeros((P, NW), np.float32)
    wpack[:F_NODE, W1_OFF:W1_OFF + H] = W1
    wpack[F_NODE:, W1_OFF + H:W1_OFF + 2 * H] = W1
    for i in range(P // H):
        wpack[H * i:H * i + H, W2_OFF + H * i:W2_OFF + H * i + H] = W2
    wpack[:, B1_OFF] = np.tile(np.asarray(b1, np.float32), P // H)
    wpack[:, B2_OFF] = np.tile(np.asarray(b2, np.float32), P // H)
    wpack[:F_COL, CW1_OFF:CW1_OFF + H] = np.asarray(col_W1, np.float32)
    wpack[:H, CB1_OFF] = np.asarray(col_b1, np.float32)

    in_maps = []
    for c in range(N_CORES):
        n0 = c * NODES_PER_CORE
        half = NODES_PER_CORE // 2
        xa = x[n0:n0 + half].T                      # [64, 12500] view
        xb = x[n0 + half:n0 + NODES_PER_CORE].T
        xT = np.ascontiguousarray(
            np.concatenate([xa, xb], axis=0), dtype=np.float32).astype(NPDT)
        wp = wpack.copy()
        wp[:F_COL, COLT_OFF:COLT_OFF + COLN] = colf[c * COLN:(c + 1) * COLN].T
        in_maps.append({"xT": xT, "wpack": wp.astype(NPDT)})
    return in_maps


def kernel(node_features, col_features, edge_index, W1, b1, W2, b2,
           node_fc_W, node_fc_b, col_W1, col_b1, col_W2, col_b2,
           fc_W, fc_b, out_W, out_b):
    global LAST_EXEC_TIME_NS, LAST_RESULTS
    # edge_index provably does not affect the output (see module docstring).
    in_maps = _prep_in_maps(node_features, col_features,
                            W1, b1, W2, b2, col_W1, col_b1)
    zeros_path = not (np.any(np.asarray(b1)) or np.any(np.asarray(b2))
                      or np.any(np.asarray(col_b1)))
    nc = _get_nc(relu1_on_dve=zeros_path)
    res = run_bass_kernel_spmd(nc, in_maps, core_ids=list(range(N_CORES)),
                               trace=PROFILE)
    LAST_EXEC_TIME_NS = res.exec_time_ns
    LAST_RESULTS = res
    outs = res.results

    node_fc_W = np.asarray(node_fc_W, np.float32)
    col_W2 = np.asarray(col_W2, np.float32)
    node_avg = np.zeros((B, 1), np.float32)
    col_avg = np.zeros((B, 1), np.float32)
    for b in range(B):
        ns = (outs[2 * b]["node_acc"].reshape(P // H, H).sum(axis=0) +
              outs[2 * b + 1]["node_acc"].reshape(P // H, H).sum(axis=0))
        cs = (outs[2 * b]["col_acc"].reshape(H) +
              outs[2 * b + 1]["col_acc"].reshape(H))
        node_avg[b, 0] = (ns / np.float32(N)) @ node_fc_W[:, 0] + \
            np.asarray(node_fc_b, np.float32)[0]
        col_avg[b, 0] = (cs / np.float32(C)) @ col_W2[:, 0] + \
            np.asarray(col_b2, np.float32)[0]

    combined = np.concatenate([node_avg, col_avg], axis=1)      # [B, 2]
    z = np.maximum(combined @ np.asarray(fc_W, np.float32) +
                   np.asarray(fc_b, np.float32), 0.0)
    out = z @ np.asarray(out_W, np.float32) + np.asarray(out_b, np.float32)
    return out.astype(np.float32)



# revision 14
# speedup vs baseline: 1.3228x; 1.3228x over previous
"""Trainium2 Bass kernel for nn_CriticNetwork (gnn_message_passing).

Mathematical simplification (verified against the reference): the
reference broadcasts edge_index to (B, 2, E) and reshapes to
(2, B*E); row-major reshape makes src == dst elementwise, so every
edge is a self-edge and with GCN normalization both GCNConv layers
collapse to plain linear layers.  Since the post-relu node/col heads
are linear, the device only needs per-core SUMS of the hidden
activations; the host applies the tiny final heads.

v2 design (vs the 35.5us v1 baseline) — driven by NTFF trace analysis:
  * fp8(e4m3) x over the wire (1.6MB/core instead of 3.2MB bf16) and
    fp8 W1 with DoubleRow matmuls: contraction 2x128 packs FOUR
    64-feature nodes per moving column pair, halving L1 PE time.
    (W2/h1 stay bf16: measured node_avg rel-err 4e-4, budget 2e-2.)
  * Row-shaped output [2,128] fp32 via an fp32 PE transpose
    (stats x I128): the v1 [128,1] column output emitted 128 4-byte
    HBM descriptors -> read-modify-write grind, ~6us of tail.
  * Few, large DMAs on three queues (sync HWDGE / scalar HWDGE /
    gpsimd SWDGE): v1's 7 chunk DMAs on one queue ran at 183GB/s.
  * Minimal instruction count: measured ~255ns dispatch overhead per
    (dependent) instruction; v1 had 522 instructions.

Per-core layout (25000 nodes): 7 psum chunks (6 x 512 cols + 53).
Each psum column holds 8 nodes (128 rows = 8 nodes x 16 hidden).
Chunk x layout [128, 4, M]: blocks (A0,A1,B0,B1); DR matmul A
consumes blocks 0-1 -> psum rows 0:64, B -> rows 64:128.
x[p, hb, m] = feat (p%64) of node(chunk_base + (2*hb + p//64)*M + m).
"""

import ml_dtypes
import numpy as np

import concourse.bacc as bacc
import concourse.mybir as mybir
import concourse.tile as tile
from concourse.bass_utils import run_bass_kernel_spmd

P = 128
N_CORES = 8
B, N, F_NODE, H = 4, 50000, 64, 16
C, F_COL = 1000, 32
NODES_PER_CORE = (B * N) // N_CORES          # 25000
COLN = (B * C) // N_CORES                    # 500 col rows per core

MM = 512                                     # psum bank cols (fp32)
NODES_PER_CHUNK = 8 * MM                     # 4096
N_FULL = NODES_PER_CORE // NODES_PER_CHUNK   # 6 full chunks
# tail: 424 nodes -> 53 cols, padded to 64 (DoubleRow AP needs the
# k-pair stride %16 == 0 and an even column count); pad nodes are
# zero and the host subtracts their bias-path contribution.
M_TAIL = 64
N_PAD = 8 * M_TAIL - (NODES_PER_CORE - N_FULL * NODES_PER_CHUNK)    # 88
CHUNK_M = [MM] * N_FULL + [M_TAIL]           # 7 chunks
N_CHUNKS = len(CHUNK_M)

# wpack (bf16) column layout
W2_OFF = 0                                   # [128, 128] blockdiag(W2 x8)
CW1_OFF = W2_OFF + P                         # [32, 16]  col_W1
COLT_OFF = CW1_OFF + H                       # [32, 500] colT
NWP = COLT_OFF + COLN                        # 644

# wpack32 (fp32) column layout
I_OFF = 0                                    # [128, 128] identity
B1_OFF = I_OFF + P                           # [128, 1] b1 tiled x8
B2_OFF = B1_OFF + 1                          # [128, 1] b2 tiled x8
NB2_OFF = B2_OFF + 1                         # [128, 1] -b2 tiled x8
CB1_OFF = NB2_OFF + 1                        # [16, 1]  col_b1
NW32 = CB1_OFF + 1                           # 132

DT = mybir.dt.bfloat16
FP8 = mybir.dt.float8e4
NPBF = ml_dtypes.bfloat16
NPF8 = ml_dtypes.float8_e4m3                 # TRN FP8_EXP4-compatible
DR = mybir.MatmulPerfMode.DoubleRow

PROFILE = False
CHECK_WAITS = True
LAST_EXEC_TIME_NS = None
LAST_RESULTS = None

_NC_CACHE = {}


def _build_nc():
    f32 = mybir.dt.float32
    Relu = mybir.ActivationFunctionType.Relu
    nc = bacc.Bacc("TRN2")

    # Two full-partition DoubleRow stationaries (the ISA rejects
    # partition-sliced PSUM outputs in DR mode): w1A covers x blocks
    # 0-1 -> psum rows 0:64 (cols 64:128 zero), w1B covers blocks
    # 2-3 -> rows 64:128; the two matmuls accumulate into one bank.
    w1t = nc.dram_tensor("w1t", [P, 2, 2 * P], FP8, kind="ExternalInput")
    # x DMA groups: chunk0 | chunks1-2 | chunks3-4 | chunk5 | chunk6
    xt0 = nc.dram_tensor("xt0", [P, 4, MM], FP8, kind="ExternalInput")
    xt1 = nc.dram_tensor("xt1", [P, 8, MM], FP8, kind="ExternalInput")
    xt2 = nc.dram_tensor("xt2", [P, 8, MM], FP8, kind="ExternalInput")
    xt3 = nc.dram_tensor("xt3", [P, 4, MM], FP8, kind="ExternalInput")
    xt4 = nc.dram_tensor("xt4", [P, 4, M_TAIL], FP8, kind="ExternalInput")
    wpack = nc.dram_tensor("wpack", [P, NWP], DT, kind="ExternalInput")
    wpack32 = nc.dram_tensor("wpack32", [P, NW32], f32, kind="ExternalInput")
    out_acc = nc.dram_tensor("out_acc", [2, P], f32, kind="ExternalOutput")

    with tile.TileContext(nc) as tc:
        with (
            tc.tile_pool(name="consts", bufs=1) as consts,
            tc.tile_pool(name="xin", bufs=1) as xin,
            tc.tile_pool(name="work", bufs=1) as work,
            tc.tile_pool(name="psum", bufs=1, space="PSUM") as psum,
        ):
            # --- input DMAs, spread over three queues -----------------
            w1s = consts.tile([P, 2, 2 * P], FP8)
            nc.scalar.dma_start(w1s[:, :, :], w1t[:, :, :])     # ACT hwdge
            x0 = xin.tile([P, 4, MM], FP8, tag="x0", name="x0")
            nc.sync.dma_start(x0[:, :, :], xt0[:, :, :])        # SP hwdge
            x1 = xin.tile([P, 8, MM], FP8, tag="x1", name="x1")
            nc.scalar.dma_start(x1[:, :, :], xt1[:, :, :])
            wp = consts.tile([P, NWP], DT)
            nc.gpsimd.dma_start(wp[:], wpack[:])                # swdge
            x2 = xin.tile([P, 8, MM], FP8, tag="x2", name="x2")
            nc.sync.dma_start(x2[:, :, :], xt2[:, :, :])
            x3 = xin.tile([P, 4, MM], FP8, tag="x3", name="x3")
            nc.gpsimd.dma_start(x3[:, :, :], xt3[:, :, :])
            x4 = xin.tile([P, 4, M_TAIL], FP8, tag="x4", name="x4")
            nc.sync.dma_start(x4[:, :, :], xt4[:, :, :])
            wp32 = consts.tile([P, NW32], f32)
            nc.gpsimd.dma_start(wp32[:], wpack32[:])

            w2_t = wp[:, W2_OFF:W2_OFF + P]
            cw1_t = wp[:F_COL, CW1_OFF:CW1_OFF + H]
            colT_t = wp[:F_COL, COLT_OFF:COLT_OFF + COLN]
            i128 = wp32[:, I_OFF:I_OFF + P]
            b1_t = wp32[:, B1_OFF:B1_OFF + 1]
            b2_t = wp32[:, B2_OFF:B2_OFF + 1]
            nb2_t = wp32[:, NB2_OFF:NB2_OFF + 1]
            cb1_t = wp32[:H, CB1_OFF:CB1_OFF + 1]

            # stats: col c = chunk-c row sums; stats_final:
            # col 0 = node totals, col 1 = col-path totals (rows 0-15).
            stats = work.tile([P, N_CHUNKS], f32)
            stats_final = work.tile([P, 2], f32)
            nc.vector.memset(stats_final[:], 0.0)

            NBUF = 3
            ps1_t = [psum.tile([P, MM], f32, tag=f"ps1_{k}", name=f"ps1_{k}")
                     for k in range(NBUF)]
            ps2_t = [psum.tile([P, MM], f32, tag=f"ps2_{k}", name=f"ps2_{k}")
                     for k in range(NBUF)]
            h1_t = [work.tile([P, MM], DT, tag=f"h1_{k}", name=f"h1_{k}")
                    for k in range(NBUF)]
            scr_t = [work.tile([P, MM], DT, tag=f"scr_{k}", name=f"scr_{k}")
                     for k in range(NBUF)]

            # chunk -> (tile, block offset in dim1)
            srcs = [(x0, 0), (x1, 0), (x1, 4), (x2, 0), (x2, 4),
                    (x3, 0), (x4, 0)]

            for c, (src, bo) in enumerate(srcs):
                M = CHUNK_M[c]
                ps1 = ps1_t[c % NBUF]
                nc.tensor.matmul(
                    ps1[:, :M], w1s[:, :, 0:P], src[:, bo:bo + 2, :M],
                    start=True, stop=False, perf_mode=DR)
                nc.tensor.matmul(
                    ps1[:, :M], w1s[:, :, P:2 * P], src[:, bo + 2:bo + 4, :M],
                    start=False, stop=True, perf_mode=DR)
                h1 = h1_t[c % NBUF]
                nc.scalar.activation(h1[:, :M], ps1[:, :M], Relu, bias=b1_t)
                ps2 = ps2_t[c % NBUF]
                nc.tensor.matmul(ps2[:, :M], w2_t, h1[:, :M],
                                 start=True, stop=True)
                scr = scr_t[c % NBUF]
                # relu(x + b2) = max(x, -b2) + b2 -- the DVE's second ALU
                # stage does not apply `max`, so keep max in stage 0.
                nc.vector.tensor_scalar(
                    scr[:, :M], ps2[:, :M], nb2_t, b2_t,
                    mybir.AluOpType.max, mybir.AluOpType.add,
                    accum_out=stats[:, c:c + 1])

            # column-features path: h = relu(colT.T @ col_W1 + col_b1)
            psc = psum.tile([H, COLN], f32, tag="psc")
            nc.tensor.matmul(psc[:, :], cw1_t, colT_t, start=True, stop=True)
            colscr = work.tile([H, COLN], f32)
            nc.scalar.activation(colscr[:], psc[:], Relu, bias=cb1_t,
                                 accum_out=stats_final[:H, 1:2])

            # node totals, then transpose [128,2] -> [2,128] on the PE
            nc.vector.tensor_reduce(stats_final[:, 0:1], stats[:],
                                    axis=mybir.AxisListType.X,
                                    op=mybir.AluOpType.add)
            ptr = psum.tile([2, P], f32, tag="ptr")
            nc.tensor.transpose(ptr[:, :], stats_final[:, 0:2], i128)
            row = work.tile([2, P], f32)
            nc.vector.tensor_copy(row[:], ptr[:])
            nc.sync.dma_start(out_acc[:], row[:])

    nc.finalize()

    if CHECK_WAITS:
        for blk in nc.m.functions[0].blocks:
            for inst in blk.instructions:
                si = inst.sync_info
                nwait = len(si.on_wait) if si and si.on_wait else 0
                limit = 2 if type(inst).__name__ in (
                    "InstEventSemaphore", "InstDrain", "InstDMACopy") else 1
                assert nwait <= limit, (
                    inst.name, type(inst).__name__,
                    [w.ant_name for w in si.on_wait])
    return nc


def _get_nc():
    if "nc" not in _NC_CACHE:
        _NC_CACHE["nc"] = _build_nc()
    return _NC_CACHE["nc"]


def _pack_x_core(xc):
    """xc [25000, 64] f32 -> [128, 12544] fp8 in chunked DR layout."""
    if N_PAD:
        xc = np.concatenate(
            [xc, np.zeros((N_PAD, F_NODE), xc.dtype)], axis=0)
    cols = []
    base = 0
    for M in CHUNK_M:
        nodes = xc[base:base + 8 * M]                 # [8M, 64]
        a = nodes.reshape(4, 2, M, F_NODE)            # (hb, prow, m, feat)
        cols.append(a.transpose(1, 3, 0, 2).reshape(P, 4 * M))
        base += 8 * M
    return np.concatenate(cols, axis=1)


def _prep_in_maps(node_features, col_features, W1, b1, W2, b2, col_W1, col_b1):
    f32 = np.float32
    x = np.ascontiguousarray(node_features, dtype=f32).reshape(B * N, F_NODE)
    colf = np.ascontiguousarray(col_features, dtype=f32).reshape(B * C, F_COL)
    W1 = np.asarray(W1, f32)
    W2 = np.asarray(W2, f32)

    # DoubleRow stationaries: w1X[p, k, 16q+f] = W1[p%64, f] where
    # q = 2*k + p//64 (4 nodes per moving column pair); w1A fills
    # out rows 0:64 (blocks 0-1), w1B rows 64:128 (blocks 2-3).
    w1dr = np.zeros((P, 2, 2 * P), f32)
    for k in range(2):
        for ph in range(2):
            q = 2 * k + ph
            w1dr[ph * 64:(ph + 1) * 64, k, 16 * q:16 * q + H] = W1          # A
            w1dr[ph * 64:(ph + 1) * 64, k, P + 64 + 16 * q:P + 64 + 16 * q + H] = W1  # B
    w1dr = w1dr.astype(NPF8)

    wpack = np.zeros((P, NWP), f32)
    for g in range(P // H):
        wpack[H * g:H * g + H, W2_OFF + H * g:W2_OFF + H * g + H] = W2
    wpack[:F_COL, CW1_OFF:CW1_OFF + H] = np.asarray(col_W1, f32)
    wpack = wpack.astype(NPBF)

    wpack32 = np.zeros((P, NW32), f32)
    wpack32[:, I_OFF:I_OFF + P] = np.eye(P, dtype=f32)
    wpack32[:, B1_OFF] = np.tile(np.asarray(b1, f32), P // H)
    wpack32[:, B2_OFF] = np.tile(np.asarray(b2, f32), P // H)
    wpack32[:, NB2_OFF] = -wpack32[:, B2_OFF]
    wpack32[:H, CB1_OFF] = np.asarray(col_b1, f32)

    bounds = np.cumsum([0] + [4 * M for M in CHUNK_M])  # chunk col offsets

    in_maps = []
    for core in range(N_CORES):
        n0 = core * NODES_PER_CORE
        xp = _pack_x_core(x[n0:n0 + NODES_PER_CORE]).astype(NPF8)
        wp = wpack.copy()
        wp[:F_COL, COLT_OFF:COLT_OFF + COLN] = \
            colf[core * COLN:(core + 1) * COLN].T.astype(NPBF)
        in_maps.append({
            "w1t": w1dr,
            "xt0": xp[:, bounds[0]:bounds[1]].reshape(P, 4, MM),
            "xt1": xp[:, bounds[1]:bounds[3]].reshape(P, 8, MM),
            "xt2": xp[:, bounds[3]:bounds[5]].reshape(P, 8, MM),
            "xt3": xp[:, bounds[5]:bounds[6]].reshape(P, 4, MM),
            "xt4": np.ascontiguousarray(
                xp[:, bounds[6]:bounds[7]].reshape(P, 4, M_TAIL)),
            "wpack": wp,
            "wpack32": wpack32,
        })
    return in_maps


def kernel(node_features, col_features, edge_index, W1, b1, W2, b2,
           node_fc_W, node_fc_b, col_W1, col_b1, col_W2, col_b2,
           fc_W, fc_b, out_W, out_b):
    global LAST_EXEC_TIME_NS, LAST_RESULTS
    # edge_index provably does not affect the output (see module docstring).
    in_maps = _prep_in_maps(node_features, col_features,
                            W1, b1, W2, b2, col_W1, col_b1)
    nc = _get_nc()
    res = run_bass_kernel_spmd(nc, in_maps, core_ids=list(range(N_CORES)),
                               trace=PROFILE)
    LAST_EXEC_TIME_NS = res.exec_time_ns
    LAST_RESULTS = res
    outs = res.results

    node_fc_W = np.asarray(node_fc_W, np.float32)
    col_W2 = np.asarray(col_W2, np.float32)
    # Device relu2 computes max(x, -b2) (+b2 once per chunk-reduction),
    # i.e. each chunk col = sum(relu) - (M-1)*b2; add the constant back.
    # Zero-pad nodes contribute relu(W2.T relu(b1) + b2) each; subtract.
    b2f = np.asarray(b2, np.float32)
    summ1 = np.float32(sum(M - 1 for M in CHUNK_M))
    pad_h2 = np.maximum(
        np.maximum(np.asarray(b1, np.float32), 0.0) @ np.asarray(W2, np.float32)
        + b2f, 0.0) * np.float32(N_PAD) - summ1 * b2f * np.float32(P // H)
    node_avg = np.zeros((B, 1), np.float32)
    col_avg = np.zeros((B, 1), np.float32)
    for b in range(B):
        acc = (outs[2 * b]["out_acc"].astype(np.float32) +
               outs[2 * b + 1]["out_acc"].astype(np.float32))
        ns = acc[0].reshape(P // H, H).sum(axis=0) - 2.0 * pad_h2
        cs = acc[1, :H]                                  # col feature sums
        node_avg[b, 0] = (ns / np.float32(N)) @ node_fc_W[:, 0] + \
            np.asarray(node_fc_b, np.float32)[0]
        col_avg[b, 0] = (cs / np.float32(C)) @ col_W2[:, 0] + \
            np.asarray(col_b2, np.float32)[0]

    combined = np.concatenate([node_avg, col_avg], axis=1)      # [B, 2]
    z = np.maximum(combined @ np.asarray(fc_W, np.float32) +
                   np.asarray(fc_b, np.float32), 0.0)
    out = z @ np.asarray(out_W, np.float32) + np.asarray(out_b, np.float32)
    return out.astype(np.float32)
